# revision 36
# baseline (speedup 1.0000x reference)
"""ConvLSTM cell kernel for Trainium2 (8 NeuronCores).

Sharding: data-parallel over batch B=4 x spatial split of H=64 into 2 halves
(8 shards). The recurrence prevents sharding T. Each core computes its half
with a shrinking row margin (47-t rows at step t) so no cross-core
communication is ever needed: row validity shrinks by 1 per conv step, and
16 margin rows cover all 16 steps. Bottom halves are row-flipped on the host
(x rows flipped + conv kernel dy-flipped) so a single SPMD program serves
all 8 cores.

On-core layout:
  h lives in SBUF as [128, 49, 68] bf16 "HB": partitions 64-127 hold hpad
  (1 zero pad row on top, 2 zero pad cols left, 2 right), partitions 0-63
  hold the same data shifted down one row. A companion tile "BB" holds hpad
  shifted left 1 column (partitions 0-63) and left 2 columns (64-127). A
  3x3 conv then needs only 5 matmul issues per 128-wide oc tile: 3 K=128
  issues on HB cover tap pairs (dy=0 paired with dy=1) for dx=0..2, one
  K=128 issue on BB covers (dy2,dx0)+(dy2,dx1), and one K=64 issue covers
  (dy2,dx2). x_t is added in PSUM with an identity matmul issued FIRST
  (start=True) so the step boundary has h-independent PE work.

Gate packing (host-side channel perm): tmp0 = [g(0:64); f(64:128)],
tmp1 = [i(0:64); o(64:128)]. One Tanh ACT with per-partition scale [1, 0.5]
gives [g ; s_f=tanh(z/2)]; one Sigmoid ACT gives [sig_i ; sig_o] (both
functions live in the `sigmoid_and_others` table set -> zero switches).
States c (partitions 0-63) and h (64-127) are carried UNSCALED.

State update, all in 2x/4x packed-bf16 DVE modes (engines allow an output
partition-base different from the inputs', so the cross-half hops ride on
the tensor_scalar and ACT outputs; every multi-input op is same-partition):
  f  = 0.5*s_f + 0.5      (tensor_scalar, out remapped 64:128 -> 0:64)
  u  = f * c              (TT on 0:64)
  v  = sig_i * g          (TT on 0:64)
  c' = u + v              (TT on 0:64)
  tc = tanh(c')           (ACT, out remapped 0:64 -> 64:128)
  h' = sig_o * tc         (TT on 64:128, into HB rows)
The tanh/h'/shift-copy tail of each 16-row block is software-lagged by one
block (carried across the step boundary) so the Scalar and Vector FIFOs
never ping-pong head-of-line within a block, and the next step's matmuls
start with zero exposed serial tail. The HB/BB shift copies are flat
contiguous DVE copies (wrap elements land in the zero pad columns).
"""

import sys

sys.path.insert(0, "/opt/trn_rl_repo")

import numpy as np
from ml_dtypes import bfloat16

HIDDEN = 64
T_STEPS = 16
B = 4
H = 64
W = 64
OC = 4 * HIDDEN  # 256
ROWS = 48        # per-core x rows (32 owned + 16 margin)
OWN = 32
WP = W + 4       # padded row width 68 (2 left, 2 right; keeps 4B alignment)
CL = 2           # left pad columns
HROWS = ROWS + 1  # hpad rows (1 zero row on top)

_CACHE = {}


def _build_nc():
    from concourse import bacc, mybir
    from concourse.tile import TileContext

    dt = mybir.dt
    Alu = mybir.AluOpType
    Act = mybir.ActivationFunctionType

    nc = bacc.Bacc(None, target_bir_lowering=False)

    x_in = nc.dram_tensor("x", [T_STEPS, 2, 128, ROWS * W], dt.bfloat16,
                          kind="ExternalInput")
    wp_in = nc.dram_tensor("wp", [128, 6 * 128], dt.bfloat16,
                           kind="ExternalInput")
    wb_in = nc.dram_tensor("wb", [128, 2 * 128], dt.bfloat16,
                           kind="ExternalInput")
    ws_in = nc.dram_tensor("ws", [64, 2 * 128], dt.bfloat16,
                           kind="ExternalInput")
    id_in = nc.dram_tensor("ident", [128, 128], dt.bfloat16,
                           kind="ExternalInput")
    sc_in = nc.dram_tensor("scale2", [128, 1], dt.float32,
                           kind="ExternalInput")
    h0_in = nc.dram_tensor("h0", [64, HROWS * WP], dt.bfloat16,
                           kind="ExternalInput")
    c0_in = nc.dram_tensor("c0", [64, ROWS * W], dt.bfloat16,
                           kind="ExternalInput")
    hout = nc.dram_tensor("hout", [T_STEPS, 64, OWN * W], dt.bfloat16,
                          kind="ExternalOutput")

    with TileContext(nc) as tc:
        with (
            tc.tile_pool(name="const", bufs=1) as cpool,
            tc.tile_pool(name="state", bufs=1) as spool,
            tc.tile_pool(name="xload", bufs=3) as xpool,
            tc.tile_pool(name="work", bufs=3) as wpool,
            tc.tile_pool(name="ps", bufs=2, space="PSUM") as psp,
        ):
            wp_sb = cpool.tile([128, 6 * 128], dt.bfloat16, tag="wp")
            wb_sb = cpool.tile([128, 2 * 128], dt.bfloat16, tag="wb")
            # ws lives on partitions 64-127 to match the hpad half of HB
            # (matmul requires lhsT and rhs at the same base partition)
            ws_sb = cpool.tile([128, 2 * 128], dt.bfloat16, tag="ws")
            id_sb = cpool.tile([128, 128], dt.bfloat16, tag="id")
            sc_sb = cpool.tile([128, 1], dt.float32, tag="sc")
            # Startup critical path first: the opening x-matmuls need only
            # the identity and the first 16-row x chunk; the opening conv
            # matmuls then need h0/b0 and wp. Everything else follows.
            nc.sync.dma_start(out=id_sb[:], in_=id_in[:])

            # Pre-warm the PE while the startup DMAs are in flight: ~4us of
            # dummy matmuls on memset zeros satisfies the HAM activity
            # window, so the real work starts at 2.4 GHz instead of 1.2.
            wsrc = cpool.tile([128, 512], dt.bfloat16, tag="warmsrc")
            nc.vector.memset(wsrc[:], 0.0)
            warm = psp.tile([128, 1024], dt.float32, tag="tmp0", name="tmp0")
            for k in range(10):
                nc.tensor.matmul(warm[:, (k % 2) * 512: (k % 2) * 512 + 512],
                                 lhsT=wsrc[:, :128], rhs=wsrc[:],
                                 start=True, stop=True)

            # h buffers (ping-pong across steps) and cell state c.
            # bb: the dy=2 companion of hb -- partitions 64-127 mirror hpad,
            # partitions 0-63 hold hpad shifted LEFT one column, so one
            # K=128 matmul covers the (dy2,dx0)+(dy2,dx1) tap pair.
            hb = [
                spool.tile([128, HROWS, WP], dt.bfloat16, tag="hb0", name="hb0"),
                spool.tile([128, HROWS, WP], dt.bfloat16, tag="hb1", name="hb1"),
            ]
            bb = [
                spool.tile([128, HROWS, WP], dt.bfloat16, tag="bb0", name="bb0"),
                spool.tile([128, HROWS, WP], dt.bfloat16, tag="bb1", name="bb1"),
            ]
            cs = spool.tile([128, ROWS * W], dt.bfloat16, tag="cs")
            # step 0 is computed on the host (it depends only on x[0]); the
            # device starts at t=1 with h0/c0 and the pre-shifted companion
            # layouts DMA'd in. Only the t=1-written buffers need zeroed
            # pads.
            xt1 = []
            for half in range(2):
                xti = xpool.tile([128, ROWS * W], dt.bfloat16,
                                 tag=f"x{half}", name=f"x{half}")
                nc.sync.dma_start(out=xti[:, : 16 * W],
                                  in_=x_in[1, half][:, : 16 * W])
                xt1.append(xti)
            h1f = hb[1][64:128].rearrange("p a b -> p (a b)")
            nc.sync.dma_start(out=h1f, in_=h0_in[:])
            nc.sync.dma_start(out=wp_sb[:], in_=wp_in[:])
            nc.sync.dma_start(out=wb_sb[:], in_=wb_in[:])
            nc.sync.dma_start(out=sc_sb[:], in_=sc_in[:])
            nc.sync.dma_start(out=ws_sb[64:128, :], in_=ws_in[:])
            nc.sync.dma_start(out=cs[0:64, :], in_=c0_in[:])
            # derive the shifted companions on the (startup-idle) DVE: the
            # hb row-shift and the two bb column-shifts, same flat patterns
            # as the steady-state tails. The last tile row is left unwritten
            # by the shifts; t=1 reads at most hpad row 47, so it is never
            # consumed.
            L = HROWS * WP
            nc.vector.tensor_copy(
                hb[1][0:64, 0: HROWS - 1, :],
                hb[1][64:128, 1: HROWS, :])
            nc.vector.tensor_copy(
                bb[1][0:64].rearrange("p a b -> p (a b)")[:, 0: L - 2],
                h1f[:, 1: L - 1])
            nc.vector.tensor_copy(
                bb[1][64:128].rearrange("p a b -> p (a b)")[:, 0: L - 2],
                h1f[:, 2: L])
            nc.vector.memset(hb[0][:], 0.0)
            nc.gpsimd.memset(bb[0][:], 0.0)

            def emit_tail(pend):
                """Lagged block tail: tc = tanh(c'); h' = sig_o*tc; copies."""
                p_hbw, p_bbw, y0, rows, Nb, sio, tch, last_t, step_last = pend
                # c' lives on partitions 0-63; ACT's output partition remap
                # carries tanh(c') over to 64-127 where sig_o lives
                cseg = cs[0:64, y0 * W: y0 * W + Nb]
                nc.scalar.activation(tch[64:128, :Nb], cseg, Act.Tanh,
                                     scale=1.0)
                nc.vector.tensor_tensor(
                    p_hbw[64:128, 1 + y0: 1 + y0 + rows, CL: CL + W],
                    sio[64:128, :Nb], tch[64:128, :Nb], Alu.mult)
                if not last_t:
                    # shifted copy for next step's K=128 pairs: lo[r] = hi[r+1]
                    nc.vector.tensor_copy(
                        p_hbw[0:64, y0: y0 + rows, :],
                        p_hbw[64:128, y0 + 1: y0 + 1 + rows, :])
                    # bb companion for the dy=2 (dx0,dx1) pair: partitions
                    # 0-63 = hpad shifted left 1 col, 64-127 = shifted left
                    # 2 cols. Flat contiguous DVE copies (2x / 4x modes);
                    # the wrap elements land in hpad's zero pad columns.
                    # (SBUF->SBUF DMA was tried and loses: completion
                    # latency ~3.5us plus coalesced waits stall the PE.)
                    base = (1 + y0) * WP
                    ln = rows * WP
                    hflat = p_hbw[64:128].rearrange("p a b -> p (a b)")
                    blo = p_bbw[0:64].rearrange("p a b -> p (a b)")
                    bhi = p_bbw[64:128].rearrange("p a b -> p (a b)")
                    nc.vector.tensor_copy(
                        blo[:, base: base + ln],
                        hflat[:, base + 1: base + 1 + ln])
                    nc.vector.tensor_copy(
                        bhi[:, base: base + ln],
                        hflat[:, base + 2: base + 2 + ln])

            pend = None      # one-block-lagged tail state
            pend_out = None  # deferred hout DMA: (t, hbw)
            for t in range(1, T_STEPS):
                R = 47 - t  # output rows this step
                hbr = hb[t % 2]
                hbw = hb[(t + 1) % 2]
                bbr = bb[t % 2]
                bbw = bb[(t + 1) % 2]

                if t == 1:
                    # tiles pre-allocated and first chunk pre-loaded above;
                    # fetch the remaining rows
                    xt = xt1
                    for half in range(2):
                        nc.sync.dma_start(
                            out=xt[half][:, 16 * W: R * W],
                            in_=x_in[t, half][:, 16 * W: R * W])
                else:
                    xt = []
                    for half in range(2):
                        xti = xpool.tile([128, ROWS * W], dt.bfloat16,
                                         tag=f"x{half}", name=f"x{half}")
                        nc.sync.dma_start(out=xti[:, : R * W],
                                          in_=x_in[t, half][:, : R * W])
                        xt.append(xti)

                BS = 16
                nblk = (R + BS - 1) // BS
                for bi in range(nblk):
                    y0 = bi * BS
                    rows = min(BS, R - y0)
                    Nb = rows * W

                    tmp = [psp.tile([128, 1024], dt.float32, tag="tmp0",
                                    name="tmp0"),
                           psp.tile([128, 1024], dt.float32, tag="tmp1",
                                    name="tmp1")]
                    nsub = (rows + 7) // 8
                    # x injection first: h-independent PE work at block entry
                    for tau in range(2):
                        for sub in range(nsub):
                            yy = y0 + sub * 8
                            sr = min(8, rows - sub * 8)
                            n = sr * W
                            nc.tensor.matmul(
                                tmp[tau][:, sub * 512: sub * 512 + n],
                                lhsT=id_sb[:],
                                rhs=xt[tau][:, yy * W: yy * W + n],
                                start=True, stop=False)
                    if True:
                        # tap-outer order: each lhsT is loaded once per
                        # (tau, block) and streams both subs back-to-back,
                        # halving LDWEIGHTS traffic. wb last: its bb-copy
                        # dependency has the least slack.
                        for tau in range(2):
                            subs = []
                            for sub in range(nsub):
                                yy = y0 + sub * 8
                                sr = min(8, rows - sub * 8)
                                n = sr * W
                                subs.append(
                                    (yy, sr,
                                     tmp[tau][:, sub * 512: sub * 512 + n]))
                            for d in range(3):
                                for yy, sr, out_ap in subs:
                                    nc.tensor.matmul(
                                        out_ap,
                                        lhsT=wp_sb[:, (tau * 3 + d) * 128:
                                                   (tau * 3 + d + 1) * 128],
                                        rhs=hbr[:, yy: yy + sr,
                                                d + 1: d + 1 + W],
                                        start=False, stop=False)
                            # dy=2, dx2 single K=64 tap from hpad
                            for yy, sr, out_ap in subs:
                                nc.tensor.matmul(
                                    out_ap,
                                    lhsT=ws_sb[64:128, tau * 128:
                                               (tau + 1) * 128],
                                    rhs=hbr[64:128, yy + 2: yy + 2 + sr,
                                            3: 3 + W],
                                    start=False, stop=False)
                            # dy=2: (dx0,dx1) pair via the bb companion
                            for yy, sr, out_ap in subs:
                                nc.tensor.matmul(
                                    out_ap,
                                    lhsT=wb_sb[:, tau * 128:
                                               (tau + 1) * 128],
                                    rhs=bbr[:, yy + 2: yy + 2 + sr,
                                            0: W],
                                    start=False, stop=True)

                    # lagged tail from the previous block (possibly the last
                    # block of the previous step), then any deferred hout DMA
                    if pend is not None:
                        emit_tail(pend)
                        pend = None
                    if pend_out is not None:
                        pt, p_hbw = pend_out
                        nc.scalar.dma_start(
                            out=hout[pt],
                            in_=p_hbw[64:128, 1: 1 + OWN, CL: CL + W])
                        pend_out = None

                    # gates: tile0 = [g ; s_f=tanh(z/2)] (one Tanh act,
                    # per-partition scale), tile1 = [sig_i ; sig_o]
                    sgf = wpool.tile([128, 1024], dt.bfloat16, tag="sgf")
                    sio = wpool.tile([128, 1024], dt.bfloat16, tag="sio")
                    nc.scalar.activation(sgf[:, :Nb], tmp[0][:, :Nb],
                                         Act.Tanh, scale=sc_sb[:])
                    nc.scalar.activation(sio[:, :Nb], tmp[1][:, :Nb],
                                         Act.Sigmoid, scale=1.0)

                    # state update, all on partitions 0-63 so every op is a
                    # 2x/4x-mode TT/TS (the tensor_scalar's output partition
                    # remap brings f = 0.5*s_f + 0.5 down from 64-127):
                    # u = f*c ; v = sig_i*g ; c' = u + v
                    u = wpool.tile([128, 1024], dt.bfloat16, tag="u")
                    v = wpool.tile([128, 1024], dt.bfloat16, tag="v")
                    w1 = wpool.tile([128, 1024], dt.bfloat16, tag="w1")
                    tch = wpool.tile([128, 1024], dt.bfloat16, tag="tch")
                    cseg = cs[0:64, y0 * W: y0 * W + Nb]
                    nc.vector.tensor_scalar(
                        w1[0:64, :Nb], sgf[64:128, :Nb], 0.5, 0.5,
                        Alu.mult, Alu.add)
                    nc.vector.tensor_tensor(
                        u[0:64, :Nb], w1[0:64, :Nb], cseg, Alu.mult)
                    nc.vector.tensor_tensor(
                        v[0:64, :Nb], sio[0:64, :Nb], sgf[0:64, :Nb],
                        Alu.mult)
                    nc.vector.tensor_tensor(
                        cseg, u[0:64, :Nb], v[0:64, :Nb], Alu.add)

                    pend = (hbw, bbw, y0, rows, Nb, sio, tch, t == T_STEPS - 1,
                            bi == nblk - 1)

                pend_out = (t, hbw)

            # flush the final block's tail and the last output store
            emit_tail(pend)
            pt, p_hbw = pend_out
            nc.scalar.dma_start(out=hout[pt],
                                in_=p_hbw[64:128, 1: 1 + OWN, CL: CL + W])

    nc.finalize()
    return nc


def _prep_inputs(x, w_h2h):
    """Build per-core input maps. Cores: core = b*2 + half."""
    # gate order in PSUM tiles: tile0 = [g(0:64); f(64:128)],
    # tile1 = [i(0:64); o(64:128)]; reference channel order is [i, f, o, g]
    perm = np.concatenate([np.arange(192, 256), np.arange(64, 128),
                           np.arange(0, 64), np.arange(128, 192)])
    w_eff = w_h2h.astype(np.float32)[perm]  # [256, 64, 3, 3]

    def pack_w(weff):
        # wp lhsT rows: 0-63 multiply the row-shifted h copy (tap dy=1),
        # 64-127 multiply hpad (tap dy=0). wb covers dy=2 x (dx0, dx1) via
        # the bb companion (lo = hpad shifted left 1 col -> tap dx1);
        # ws covers the single remaining (dy2, dx2) tap.
        wp = np.zeros((128, 2, 3, 128), np.float32)
        wb = np.zeros((128, 2, 128), np.float32)
        ws = np.zeros((64, 2, 128), np.float32)
        for tau in range(2):
            blk = weff[tau * 128: (tau + 1) * 128]  # [128oc, 64ic, 3, 3]
            for d in range(3):
                wp[0:64, tau, d, :] = blk[:, :, 1, d].T
                wp[64:128, tau, d, :] = blk[:, :, 0, d].T
            wb[0:64, tau, :] = blk[:, :, 2, 0].T
            wb[64:128, tau, :] = blk[:, :, 2, 1].T
            ws[:, tau, :] = blk[:, :, 2, 2].T
        return (wp.reshape(128, 6 * 128).astype(bfloat16),
                wb.reshape(128, 2 * 128).astype(bfloat16),
                ws.reshape(64, 2 * 128).astype(bfloat16))

    wp_top, wb_top, ws_top = pack_w(w_eff)
    wp_bot, wb_bot, ws_bot = pack_w(w_eff[:, :, ::-1, :])
    ident = np.eye(128, dtype=np.float32).astype(bfloat16)
    scale2 = np.concatenate([np.ones((64, 1), np.float32),
                             np.full((64, 1), 0.5, np.float32)])

    # step 0 on the host: c0 = sig(x_i)*tanh(x_g), h0 = sig(x_o)*tanh(c0)
    # (f-gate is irrelevant at t=0 since c starts at zero)
    x0 = x[0].astype(np.float32)  # [B, 256, H, W], original channel order
    sig_i = 1.0 / (1.0 + np.exp(-x0[:, 0:64]))
    sig_o = 1.0 / (1.0 + np.exp(-x0[:, 128:192]))
    g0 = np.tanh(x0[:, 192:256])
    c0f = sig_i * g0
    h0f = sig_o * np.tanh(c0f)  # [B, 64, H, W]

    def state_maps(hsl, csl):
        hpad = np.zeros((64, HROWS, WP), np.float32)
        hpad[:, 1: 1 + ROWS, CL: CL + W] = hsl
        return (hpad.reshape(64, HROWS * WP).astype(bfloat16),
                csl.reshape(64, ROWS * W).astype(bfloat16))

    xp = x[:, :, perm]  # [T, B, 256, H, W] permuted channels
    in_maps = []
    for b in range(B):
        for half in range(2):
            if half == 0:
                xs = xp[:, b, :, 0:ROWS, :]
                h0m, c0m = state_maps(h0f[b, :, 0:ROWS],
                                      c0f[b, :, 0:ROWS])
            else:
                xs = xp[:, b, :, H - ROWS:, :][:, :, ::-1, :]
                h0m, c0m = state_maps(h0f[b, :, H - ROWS:][:, ::-1],
                                      c0f[b, :, H - ROWS:][:, ::-1])
            xs = np.ascontiguousarray(xs).astype(bfloat16)
            xs = xs.reshape(T_STEPS, 2, 128, ROWS * W)
            in_maps.append({
                "x": xs,
                "wp": wp_top if half == 0 else wp_bot,
                "wb": wb_top if half == 0 else wb_bot,
                "ws": ws_top if half == 0 else ws_bot,
                "ident": ident,
                "scale2": scale2,
                "h0": h0m,
                "c0": c0m,
            })
    return in_maps, h0f


def kernel(x, w_h2h):
    from concourse import bass_utils

    if "nc" not in _CACHE:
        _CACHE["nc"] = _build_nc()
    nc = _CACHE["nc"]

    in_maps, h0f = _prep_inputs(np.asarray(x), np.asarray(w_h2h))
    res = bass_utils.run_bass_kernel_spmd(nc, in_maps,
                                          core_ids=list(range(8)))
    _CACHE["last_results"] = res

    out = np.zeros((T_STEPS, B, HIDDEN, H, W), np.float32)
    out[0] = h0f  # step 0 computed on the host
    for b in range(B):
        for half in range(2):
            core = b * 2 + half
            hs = res.results[core]["hout"].astype(np.float32)
            hs = hs.reshape(T_STEPS, HIDDEN, OWN, W)[1:]
            if half == 0:
                out[1:, b, :, 0:OWN, :] = hs
            else:
                out[1:, b, :, OWN:, :] = hs[:, :, ::-1, :]
    return out


# revision 37
# speedup vs baseline: 1.0038x; 1.0038x over previous
"""ConvLSTM cell kernel for Trainium2 (8 NeuronCores).

Sharding: data-parallel over batch B=4 x spatial split of H=64 into 2 halves
(8 shards). The recurrence prevents sharding T. Each core computes its half
with a shrinking row margin (47-t rows at step t) so no cross-core
communication is ever needed: row validity shrinks by 1 per conv step, and
16 margin rows cover all 16 steps. Bottom halves are row-flipped on the host
(x rows flipped + conv kernel dy-flipped) so a single SPMD program serves
all 8 cores.

On-core layout:
  h lives in SBUF as [128, 49, 68] bf16 "HB": partitions 64-127 hold hpad
  (1 zero pad row on top, 2 zero pad cols left, 2 right), partitions 0-63
  hold the same data shifted down one row. A companion tile "BB" holds hpad
  shifted left 1 column (partitions 0-63) and left 2 columns (64-127). A
  3x3 conv then needs only 5 matmul issues per 128-wide oc tile: 3 K=128
  issues on HB cover tap pairs (dy=0 paired with dy=1) for dx=0..2, one
  K=128 issue on BB covers (dy2,dx0)+(dy2,dx1), and one K=64 issue covers
  (dy2,dx2). x_t is added in PSUM with an identity matmul issued FIRST
  (start=True) so the step boundary has h-independent PE work.

Gate packing (host-side channel perm): tmp0 = [g(0:64); f(64:128)],
tmp1 = [i(0:64); o(64:128)]. One Tanh ACT with per-partition scale [1, 0.5]
gives [g ; s_f=tanh(z/2)]; one Sigmoid ACT gives [sig_i ; sig_o] (both
functions live in the `sigmoid_and_others` table set -> zero switches).
States c (partitions 0-63) and h (64-127) are carried UNSCALED.

State update, all in 2x/4x packed-bf16 DVE modes (engines allow an output
partition-base different from the inputs', so the cross-half hops ride on
the tensor_scalar and ACT outputs; every multi-input op is same-partition):
  f  = 0.5*s_f + 0.5      (tensor_scalar, out remapped 64:128 -> 0:64)
  u  = f * c              (TT on 0:64)
  v  = sig_i * g          (TT on 0:64)
  c' = u + v              (TT on 0:64)
  tc = tanh(c')           (ACT, out remapped 0:64 -> 64:128)
  h' = sig_o * tc         (TT on 64:128, into HB rows)
The tanh/h'/shift-copy tail of each 16-row block is software-lagged by one
block (carried across the step boundary) so the Scalar and Vector FIFOs
never ping-pong head-of-line within a block, and the next step's matmuls
start with zero exposed serial tail. The HB/BB shift copies are flat
contiguous DVE copies (wrap elements land in the zero pad columns).
"""

import sys

sys.path.insert(0, "/opt/trn_rl_repo")

import numpy as np
from ml_dtypes import bfloat16

HIDDEN = 64
T_STEPS = 16
B = 4
H = 64
W = 64
OC = 4 * HIDDEN  # 256
ROWS = 48        # per-core x rows (32 owned + 16 margin)
OWN = 32
WP = W + 4       # padded row width 68 (2 left, 2 right; keeps 4B alignment)
CL = 2           # left pad columns
HROWS = ROWS + 1  # hpad rows (1 zero row on top)

_CACHE = {}


def _build_nc():
    from concourse import bacc, mybir
    from concourse.tile import TileContext

    dt = mybir.dt
    Alu = mybir.AluOpType
    Act = mybir.ActivationFunctionType

    nc = bacc.Bacc(None, target_bir_lowering=False)

    x_in = nc.dram_tensor("x", [T_STEPS, 2, 128, ROWS * W], dt.bfloat16,
                          kind="ExternalInput")
    wp_in = nc.dram_tensor("wp", [128, 6 * 128], dt.bfloat16,
                           kind="ExternalInput")
    wb_in = nc.dram_tensor("wb", [128, 2 * 128], dt.bfloat16,
                           kind="ExternalInput")
    ws_in = nc.dram_tensor("ws", [64, 2 * 128], dt.bfloat16,
                           kind="ExternalInput")
    id_in = nc.dram_tensor("ident", [128, 128], dt.bfloat16,
                           kind="ExternalInput")
    sc_in = nc.dram_tensor("scale2", [128, 1], dt.float32,
                           kind="ExternalInput")
    h0_in = nc.dram_tensor("h0", [64, HROWS * WP], dt.bfloat16,
                           kind="ExternalInput")
    c0_in = nc.dram_tensor("c0", [64, ROWS * W], dt.bfloat16,
                           kind="ExternalInput")
    hout = nc.dram_tensor("hout", [T_STEPS, 64, OWN * W], dt.bfloat16,
                          kind="ExternalOutput")

    with TileContext(nc) as tc:
        with (
            tc.tile_pool(name="const", bufs=1) as cpool,
            tc.tile_pool(name="state", bufs=1) as spool,
            tc.tile_pool(name="xload", bufs=3) as xpool,
            tc.tile_pool(name="work", bufs=3) as wpool,
            tc.tile_pool(name="ps", bufs=2, space="PSUM") as psp,
        ):
            wp_sb = cpool.tile([128, 6 * 128], dt.bfloat16, tag="wp")
            wb_sb = cpool.tile([128, 2 * 128], dt.bfloat16, tag="wb")
            # ws lives on partitions 64-127 to match the hpad half of HB
            # (matmul requires lhsT and rhs at the same base partition)
            ws_sb = cpool.tile([128, 2 * 128], dt.bfloat16, tag="ws")
            id_sb = cpool.tile([128, 128], dt.bfloat16, tag="id")
            sc_sb = cpool.tile([128, 1], dt.float32, tag="sc")
            # Startup critical path first: the opening x-matmuls need only
            # the identity and the first 16-row x chunk; the opening conv
            # matmuls then need h0/b0 and wp. Everything else follows.
            nc.sync.dma_start(out=id_sb[:], in_=id_in[:])

            # h buffers (ping-pong across steps) and cell state c.
            # bb: the dy=2 companion of hb -- partitions 64-127 mirror hpad,
            # partitions 0-63 hold hpad shifted LEFT one column, so one
            # K=128 matmul covers the (dy2,dx0)+(dy2,dx1) tap pair.
            hb = [
                spool.tile([128, HROWS, WP], dt.bfloat16, tag="hb0", name="hb0"),
                spool.tile([128, HROWS, WP], dt.bfloat16, tag="hb1", name="hb1"),
            ]
            bb = [
                spool.tile([128, HROWS, WP], dt.bfloat16, tag="bb0", name="bb0"),
                spool.tile([128, HROWS, WP], dt.bfloat16, tag="bb1", name="bb1"),
            ]
            cs = spool.tile([128, ROWS * W], dt.bfloat16, tag="cs")
            # step 0 is computed on the host (it depends only on x[0]); the
            # device starts at t=1 with h0/c0 and the pre-shifted companion
            # layouts DMA'd in. Only the t=1-written buffers need zeroed
            # pads.
            xt1 = []
            for half in range(2):
                xti = xpool.tile([128, ROWS * W], dt.bfloat16,
                                 tag=f"x{half}", name=f"x{half}")
                nc.sync.dma_start(out=xti[:, : 16 * W],
                                  in_=x_in[1, half][:, : 16 * W])
                xt1.append(xti)
            h1f = hb[1][64:128].rearrange("p a b -> p (a b)")
            nc.sync.dma_start(out=h1f, in_=h0_in[:])
            nc.sync.dma_start(out=wp_sb[:], in_=wp_in[:])
            nc.sync.dma_start(out=wb_sb[:], in_=wb_in[:])
            nc.sync.dma_start(out=sc_sb[:], in_=sc_in[:])
            nc.sync.dma_start(out=ws_sb[64:128, :], in_=ws_in[:])
            nc.sync.dma_start(out=cs[0:64, :], in_=c0_in[:])
            # derive the shifted companions on the (startup-idle) DVE: the
            # hb row-shift and the two bb column-shifts, same flat patterns
            # as the steady-state tails. The last tile row is left unwritten
            # by the shifts; t=1 reads at most hpad row 47, so it is never
            # consumed.
            L = HROWS * WP
            nc.vector.tensor_copy(
                hb[1][0:64, 0: HROWS - 1, :],
                hb[1][64:128, 1: HROWS, :])
            nc.vector.tensor_copy(
                bb[1][0:64].rearrange("p a b -> p (a b)")[:, 0: L - 2],
                h1f[:, 1: L - 1])
            nc.vector.tensor_copy(
                bb[1][64:128].rearrange("p a b -> p (a b)")[:, 0: L - 2],
                h1f[:, 2: L])
            nc.vector.memset(hb[0][:], 0.0)
            nc.gpsimd.memset(bb[0][:], 0.0)

            def emit_tail(pend):
                """Lagged block tail: tc = tanh(c'); h' = sig_o*tc; copies."""
                p_hbw, p_bbw, y0, rows, Nb, sio, tch, last_t, step_last = pend
                # c' lives on partitions 0-63; ACT's output partition remap
                # carries tanh(c') over to 64-127 where sig_o lives
                cseg = cs[0:64, y0 * W: y0 * W + Nb]
                nc.scalar.activation(tch[64:128, :Nb], cseg, Act.Tanh,
                                     scale=1.0)
                nc.vector.tensor_tensor(
                    p_hbw[64:128, 1 + y0: 1 + y0 + rows, CL: CL + W],
                    sio[64:128, :Nb], tch[64:128, :Nb], Alu.mult)
                if not last_t:
                    # shifted copy for next step's K=128 pairs: lo[r] = hi[r+1]
                    nc.vector.tensor_copy(
                        p_hbw[0:64, y0: y0 + rows, :],
                        p_hbw[64:128, y0 + 1: y0 + 1 + rows, :])
                    # bb companion for the dy=2 (dx0,dx1) pair: partitions
                    # 0-63 = hpad shifted left 1 col, 64-127 = shifted left
                    # 2 cols. Flat contiguous DVE copies (2x / 4x modes);
                    # the wrap elements land in hpad's zero pad columns.
                    # (SBUF->SBUF DMA was tried and loses: completion
                    # latency ~3.5us plus coalesced waits stall the PE.)
                    base = (1 + y0) * WP
                    ln = rows * WP
                    hflat = p_hbw[64:128].rearrange("p a b -> p (a b)")
                    blo = p_bbw[0:64].rearrange("p a b -> p (a b)")
                    bhi = p_bbw[64:128].rearrange("p a b -> p (a b)")
                    nc.vector.tensor_copy(
                        blo[:, base: base + ln],
                        hflat[:, base + 1: base + 1 + ln])
                    nc.vector.tensor_copy(
                        bhi[:, base: base + ln],
                        hflat[:, base + 2: base + 2 + ln])

            pend = None      # one-block-lagged tail state
            pend_out = None  # deferred hout DMA: (t, hbw)
            for t in range(1, T_STEPS):
                R = 47 - t  # output rows this step
                hbr = hb[t % 2]
                hbw = hb[(t + 1) % 2]
                bbr = bb[t % 2]
                bbw = bb[(t + 1) % 2]

                if t == 1:
                    # tiles pre-allocated and first chunk pre-loaded above;
                    # fetch the remaining rows
                    xt = xt1
                    for half in range(2):
                        nc.sync.dma_start(
                            out=xt[half][:, 16 * W: R * W],
                            in_=x_in[t, half][:, 16 * W: R * W])
                else:
                    xt = []
                    for half in range(2):
                        xti = xpool.tile([128, ROWS * W], dt.bfloat16,
                                         tag=f"x{half}", name=f"x{half}")
                        nc.sync.dma_start(out=xti[:, : R * W],
                                          in_=x_in[t, half][:, : R * W])
                        xt.append(xti)

                BS = 16
                nblk = (R + BS - 1) // BS
                for bi in range(nblk):
                    y0 = bi * BS
                    rows = min(BS, R - y0)
                    Nb = rows * W

                    tmp = [psp.tile([128, 1024], dt.float32, tag="tmp0",
                                    name="tmp0"),
                           psp.tile([128, 1024], dt.float32, tag="tmp1",
                                    name="tmp1")]
                    nsub = (rows + 7) // 8
                    # x injection first: h-independent PE work at block entry
                    for tau in range(2):
                        for sub in range(nsub):
                            yy = y0 + sub * 8
                            sr = min(8, rows - sub * 8)
                            n = sr * W
                            nc.tensor.matmul(
                                tmp[tau][:, sub * 512: sub * 512 + n],
                                lhsT=id_sb[:],
                                rhs=xt[tau][:, yy * W: yy * W + n],
                                start=True, stop=False)
                    if True:
                        # tap-outer order: each lhsT is loaded once per
                        # (tau, block) and streams both subs back-to-back,
                        # halving LDWEIGHTS traffic. wb last: its bb-copy
                        # dependency has the least slack.
                        for tau in range(2):
                            subs = []
                            for sub in range(nsub):
                                yy = y0 + sub * 8
                                sr = min(8, rows - sub * 8)
                                n = sr * W
                                subs.append(
                                    (yy, sr,
                                     tmp[tau][:, sub * 512: sub * 512 + n]))
                            for d in range(3):
                                for yy, sr, out_ap in subs:
                                    nc.tensor.matmul(
                                        out_ap,
                                        lhsT=wp_sb[:, (tau * 3 + d) * 128:
                                                   (tau * 3 + d + 1) * 128],
                                        rhs=hbr[:, yy: yy + sr,
                                                d + 1: d + 1 + W],
                                        start=False, stop=False)
                            # dy=2, dx2 single K=64 tap from hpad
                            for yy, sr, out_ap in subs:
                                nc.tensor.matmul(
                                    out_ap,
                                    lhsT=ws_sb[64:128, tau * 128:
                                               (tau + 1) * 128],
                                    rhs=hbr[64:128, yy + 2: yy + 2 + sr,
                                            3: 3 + W],
                                    start=False, stop=False)
                            # dy=2: (dx0,dx1) pair via the bb companion
                            for yy, sr, out_ap in subs:
                                nc.tensor.matmul(
                                    out_ap,
                                    lhsT=wb_sb[:, tau * 128:
                                               (tau + 1) * 128],
                                    rhs=bbr[:, yy + 2: yy + 2 + sr,
                                            0: W],
                                    start=False, stop=True)

                    # lagged tail from the previous block (possibly the last
                    # block of the previous step), then any deferred hout DMA
                    if pend is not None:
                        emit_tail(pend)
                        pend = None
                    if pend_out is not None:
                        pt, p_hbw = pend_out
                        nc.scalar.dma_start(
                            out=hout[pt],
                            in_=p_hbw[64:128, 1: 1 + OWN, CL: CL + W])
                        pend_out = None

                    # gates: tile0 = [g ; s_f=tanh(z/2)] (one Tanh act,
                    # per-partition scale), tile1 = [sig_i ; sig_o]
                    sgf = wpool.tile([128, 1024], dt.bfloat16, tag="sgf")
                    sio = wpool.tile([128, 1024], dt.bfloat16, tag="sio")
                    nc.scalar.activation(sgf[:, :Nb], tmp[0][:, :Nb],
                                         Act.Tanh, scale=sc_sb[:])
                    nc.scalar.activation(sio[:, :Nb], tmp[1][:, :Nb],
                                         Act.Sigmoid, scale=1.0)

                    # state update, all on partitions 0-63 so every op is a
                    # 2x/4x-mode TT/TS (the tensor_scalar's output partition
                    # remap brings f = 0.5*s_f + 0.5 down from 64-127):
                    # u = f*c ; v = sig_i*g ; c' = u + v
                    u = wpool.tile([128, 1024], dt.bfloat16, tag="u")
                    v = wpool.tile([128, 1024], dt.bfloat16, tag="v")
                    w1 = wpool.tile([128, 1024], dt.bfloat16, tag="w1")
                    tch = wpool.tile([128, 1024], dt.bfloat16, tag="tch")
                    cseg = cs[0:64, y0 * W: y0 * W + Nb]
                    nc.vector.tensor_scalar(
                        w1[0:64, :Nb], sgf[64:128, :Nb], 0.5, 0.5,
                        Alu.mult, Alu.add)
                    nc.vector.tensor_tensor(
                        u[0:64, :Nb], w1[0:64, :Nb], cseg, Alu.mult)
                    nc.vector.tensor_tensor(
                        v[0:64, :Nb], sio[0:64, :Nb], sgf[0:64, :Nb],
                        Alu.mult)
                    nc.vector.tensor_tensor(
                        cseg, u[0:64, :Nb], v[0:64, :Nb], Alu.add)

                    pend = (hbw, bbw, y0, rows, Nb, sio, tch, t == T_STEPS - 1,
                            bi == nblk - 1)

                pend_out = (t, hbw)

            # flush the final block's tail and the last output store
            emit_tail(pend)
            pt, p_hbw = pend_out
            nc.scalar.dma_start(out=hout[pt],
                                in_=p_hbw[64:128, 1: 1 + OWN, CL: CL + W])

    nc.finalize()
    return nc


def _prep_inputs(x, w_h2h):
    """Build per-core input maps. Cores: core = b*2 + half."""
    # gate order in PSUM tiles: tile0 = [g(0:64); f(64:128)],
    # tile1 = [i(0:64); o(64:128)]; reference channel order is [i, f, o, g]
    perm = np.concatenate([np.arange(192, 256), np.arange(64, 128),
                           np.arange(0, 64), np.arange(128, 192)])
    w_eff = w_h2h.astype(np.float32)[perm]  # [256, 64, 3, 3]

    def pack_w(weff):
        # wp lhsT rows: 0-63 multiply the row-shifted h copy (tap dy=1),
        # 64-127 multiply hpad (tap dy=0). wb covers dy=2 x (dx0, dx1) via
        # the bb companion (lo = hpad shifted left 1 col -> tap dx1);
        # ws covers the single remaining (dy2, dx2) tap.
        wp = np.zeros((128, 2, 3, 128), np.float32)
        wb = np.zeros((128, 2, 128), np.float32)
        ws = np.zeros((64, 2, 128), np.float32)
        for tau in range(2):
            blk = weff[tau * 128: (tau + 1) * 128]  # [128oc, 64ic, 3, 3]
            for d in range(3):
                wp[0:64, tau, d, :] = blk[:, :, 1, d].T
                wp[64:128, tau, d, :] = blk[:, :, 0, d].T
            wb[0:64, tau, :] = blk[:, :, 2, 0].T
            wb[64:128, tau, :] = blk[:, :, 2, 1].T
            ws[:, tau, :] = blk[:, :, 2, 2].T
        return (wp.reshape(128, 6 * 128).astype(bfloat16),
                wb.reshape(128, 2 * 128).astype(bfloat16),
                ws.reshape(64, 2 * 128).astype(bfloat16))

    wp_top, wb_top, ws_top = pack_w(w_eff)
    wp_bot, wb_bot, ws_bot = pack_w(w_eff[:, :, ::-1, :])
    ident = np.eye(128, dtype=np.float32).astype(bfloat16)
    scale2 = np.concatenate([np.ones((64, 1), np.float32),
                             np.full((64, 1), 0.5, np.float32)])

    # step 0 on the host: c0 = sig(x_i)*tanh(x_g), h0 = sig(x_o)*tanh(c0)
    # (f-gate is irrelevant at t=0 since c starts at zero)
    x0 = x[0].astype(np.float32)  # [B, 256, H, W], original channel order
    sig_i = 1.0 / (1.0 + np.exp(-x0[:, 0:64]))
    sig_o = 1.0 / (1.0 + np.exp(-x0[:, 128:192]))
    g0 = np.tanh(x0[:, 192:256])
    c0f = sig_i * g0
    h0f = sig_o * np.tanh(c0f)  # [B, 64, H, W]

    def state_maps(hsl, csl):
        hpad = np.zeros((64, HROWS, WP), np.float32)
        hpad[:, 1: 1 + ROWS, CL: CL + W] = hsl
        return (hpad.reshape(64, HROWS * WP).astype(bfloat16),
                csl.reshape(64, ROWS * W).astype(bfloat16))

    xp = x[:, :, perm]  # [T, B, 256, H, W] permuted channels
    in_maps = []
    for b in range(B):
        for half in range(2):
            if half == 0:
                xs = xp[:, b, :, 0:ROWS, :]
                h0m, c0m = state_maps(h0f[b, :, 0:ROWS],
                                      c0f[b, :, 0:ROWS])
            else:
                xs = xp[:, b, :, H - ROWS:, :][:, :, ::-1, :]
                h0m, c0m = state_maps(h0f[b, :, H - ROWS:][:, ::-1],
                                      c0f[b, :, H - ROWS:][:, ::-1])
            xs = np.ascontiguousarray(xs).astype(bfloat16)
            xs = xs.reshape(T_STEPS, 2, 128, ROWS * W)
            in_maps.append({
                "x": xs,
                "wp": wp_top if half == 0 else wp_bot,
                "wb": wb_top if half == 0 else wb_bot,
                "ws": ws_top if half == 0 else ws_bot,
                "ident": ident,
                "scale2": scale2,
                "h0": h0m,
                "c0": c0m,
            })
    return in_maps, h0f


def kernel(x, w_h2h):
    from concourse import bass_utils

    if "nc" not in _CACHE:
        _CACHE["nc"] = _build_nc()
    nc = _CACHE["nc"]

    in_maps, h0f = _prep_inputs(np.asarray(x), np.asarray(w_h2h))
    res = bass_utils.run_bass_kernel_spmd(nc, in_maps,
                                          core_ids=list(range(8)))
    _CACHE["last_results"] = res

    out = np.zeros((T_STEPS, B, HIDDEN, H, W), np.float32)
    out[0] = h0f  # step 0 computed on the host
    for b in range(B):
        for half in range(2):
            core = b * 2 + half
            hs = res.results[core]["hout"].astype(np.float32)
            hs = hs.reshape(T_STEPS, HIDDEN, OWN, W)[1:]
            if half == 0:
                out[1:, b, :, 0:OWN, :] = hs
            else:
                out[1:, b, :, OWN:, :] = hs[:, :, ::-1, :]
    return out
